# revision 17
# baseline (speedup 1.0000x reference)
"""MultiHeadAttention Trainium2 Bass kernel.

Problem: B=8, H=W=32 (S=1024), C=512, 8 heads x 64 dim.
Sharding: data-parallel over batch, one batch element per NeuronCore (8 cores).

Per-core pipeline (batch b):
  Phase A (projections, float32r operands / fp32 accumulate): for x in
    {v,k,q}: DMA x [1024,512], PE-transpose to xT [c,s], W-stationary f32r
    matmuls: KT/QT [d,s] transposed (head dims on partitions), V [s,d]
    natural with a ones column appended (softmax denominator). Q/K biases
    are added during PSUM evacuation (per-partition tensor_scalar, which
    also rounds to bf16); V bias via a K=1 rank-1 matmul in the group.
  Phase B (attention, bf16 operands, per head pair): scoresT[k,q] matmuls
    (K=64, two heads of a pair at PE base partitions 0/64); exp via ACT
    straight from PSUM in [128,2,512] groups with the 1/8 scale folded in
    (no max-subtraction: scores ~N(0,1)); att@V as V_aug-stationary matmul
    accumulating over k chunks (ones column = denominator), interleaved
    with the score groups so PE fills ACT-bound gaps; PE back-transpose of
    [65, q] tiles; DVE reciprocal + per-partition scale into staged output.
    Q-projection chunks 2,3 are emitted between attention blocks to fill
    PE idle time under the ACT-bound stretch.
  Phase C: one batched 2MB output DMA.

Precision: f32r (single-pass fp32 matmul mode) for projections, bf16 for
attention operands, fp32 accumulation everywhere. Measured end-to-end
absmax relative error ~5.5e-3 against the fp64 reference.
"""
import sys

import numpy as np

if "/opt/trn_rl_repo" not in sys.path:
    sys.path.insert(0, "/opt/trn_rl_repo")

import concourse.bacc as bacc
import concourse.mybir as mybir
import concourse.tile as tile
from concourse import masks
from concourse.bass_utils import run_bass_kernel_spmd

B, HS, WS, C = 8, 32, 32, 512
S = HS * WS          # 1024
D = 512
HEADS = 8
HD = 64              # head dim
N_CORES = 8

f32 = mybir.dt.float32
f32r = mybir.dt.float32r
bf16 = mybir.dt.bfloat16
Exp = mybir.ActivationFunctionType.Exp


def build_nc():
    nc = bacc.Bacc("TRN2", target_bir_lowering=False, debug=False,
                   num_devices=N_CORES)

    x_d = {}
    w_d = {}
    b_d = {}
    for name in ("q", "k", "v"):
        x_d[name] = nc.dram_tensor(f"{name}_in", [S, C], f32, kind="ExternalInput")
        w_d[name] = nc.dram_tensor(f"W{name}", [C, D], f32, kind="ExternalInput")
        b_d[name] = nc.dram_tensor(f"b{name}", [D], f32, kind="ExternalInput")
    out_d = nc.dram_tensor("out", [S, D], f32, kind="ExternalOutput")

    with tile.TileContext(nc) as tc:
        with (
            tc.tile_pool(name="const", bufs=1) as cpool,
            tc.tile_pool(name="xin", bufs=2) as xin_pool,
            tc.tile_pool(name="wbuf", bufs=2) as w_pool,
            tc.tile_pool(name="proj", bufs=1) as proj_pool,
            tc.tile_pool(name="xT", bufs=2) as xt_pool,
            tc.tile_pool(name="att", bufs=2) as att_pool,
            tc.tile_pool(name="ot", bufs=3) as ot_pool,
            tc.tile_pool(name="ostage", bufs=1) as o_pool,
            # PSUM budget (8 banks): mix shared 2 + pss single tag 3x2 = 8
            tc.tile_pool(name="ps_mix", bufs=2, space="PSUM") as ps_mix,
            tc.tile_pool(name="ps_s", bufs=3, space="PSUM") as ps_s,
        ):
            ident_f32 = cpool.tile([128, 128], f32)
            masks.make_identity(nc, ident_f32[:])
            ones_sb = cpool.tile([128, 512], f32)
            nc.vector.memset(ones_sb[:], 1.0)
            ones_r = cpool.tile([1, 512], f32r)
            nc.vector.tensor_copy(ones_r[:], ones_sb[0:1, :])

            # Persistent projection outputs
            QT = proj_pool.tile([128, 4, S], bf16, name="QT")  # [d%128, d//128, s]
            KT = proj_pool.tile([128, 4, S], bf16, name="KT")
            # V_aug: [s%128, s//128, head, 65]; col 64 = 1.0 (denominator)
            V = proj_pool.tile([128, 8, HEADS, HD + 1], bf16, name="V")
            nc.vector.tensor_copy(
                V[:, :, :, HD:HD + 1],
                ones_sb[:, 0:64].rearrange("p (a b o) -> p a b o", a=8, b=8))
            o_stage = o_pool.tile([128, 8, D], f32, name="o_stage")

            # ---------- projection helpers ----------
            def load_and_transpose(name):
                """DMA x, W, b; PE-transpose x -> xT (f32r)."""
                x_r = x_d[name][:].rearrange("(t p) c -> p t c", p=128)
                xT = xt_pool.tile([128, 4, S], f32r, name=f"xT_{name}", tag="xT")
                x_sbs = []
                for sh in range(2):
                    x_sb = xin_pool.tile([128, 4, C], f32,
                                         name=f"x_{name}{sh}", tag="x_sb")
                    nc.sync.dma_start(x_sb[:], x_r[:, sh * 4:(sh + 1) * 4, :])
                    x_sbs.append(x_sb)
                w_sb = w_pool.tile([128, 4, D], f32, name=f"w_{name}", tag="w_sb")
                nc.sync.dma_start(
                    w_sb[:], w_d[name][:].rearrange("(cc p) d -> p cc d", p=128))
                w_r = w_pool.tile([128, 4, D], f32r, name=f"wr_{name}", tag="w_r")
                nc.vector.tensor_copy(w_r[:], w_sb[:])
                # bias as [128, 4]: b_sb[p, dt] = b[dt*128 + p]
                b_sb = w_pool.tile([128, 4], f32, name=f"b_{name}", tag="b_sb")
                nc.sync.dma_start(
                    b_sb[:], b_d[name][:].rearrange("(dt p) -> p dt", p=128))
                for sh in range(2):
                    x_sb = x_sbs[sh]
                    for ti in range(4):
                        t = sh * 4 + ti
                        pst = ps_mix.tile([128, 4, 128], f32, tag="mix",
                                         name=f"pst_{name}_{t}")
                        for cc in range(4):
                            nc.tensor.transpose(
                                pst[:, cc, :],
                                x_sb[:, ti, cc * 128:(cc + 1) * 128],
                                ident_f32[:])
                        nc.vector.tensor_copy(
                            xT[:, :, t * 128:(t + 1) * 128], pst[:])
                return w_r, b_sb, xT

            def proj_qk(tgt, w_r, b_sb, xT, dt):
                """One d-chunk of a transposed projection: tgt[:, dt, :]."""
                for qh in range(2):
                    psq = ps_mix.tile([128, 512], f32, tag="mix",
                                     name=f"psq_{dt}_{qh}")
                    for cc in range(4):
                        nc.tensor.matmul(
                            psq[:],
                            w_r[:, cc, dt * 128:(dt + 1) * 128],
                            xT[:, cc, qh * 512:(qh + 1) * 512],
                            start=(cc == 0), stop=(cc == 3))
                    # evacuate + bias add + bf16 round, on the idle ACT
                    nc.scalar.activation(
                        tgt[:, dt, qh * 512:(qh + 1) * 512], psq[:],
                        mybir.ActivationFunctionType.Identity,
                        bias=b_sb[:, dt:dt + 1])

            def proj_v(w_r, xT):
                # V bias along the free dim: rank-1 matmul into the group
                bv_sb = w_pool.tile([1, D], f32, name="bv_sb", tag="bv_sb")
                nc.sync.dma_start(
                    bv_sb[:], b_d["v"][:].rearrange("(o d) -> o d", o=1))
                bv_f = w_pool.tile([1, D], f32r, name="bv_f", tag="bv_f")
                nc.vector.tensor_copy(bv_f[:], bv_sb[:])
                for st in range(8):
                    psv = ps_mix.tile([128, 512], f32, tag="mix", name=f"psv_{st}")
                    for cc in range(4):
                        nc.tensor.matmul(
                            psv[:],
                            xT[:, cc, st * 128:(st + 1) * 128],
                            w_r[:, cc, :],
                            start=(cc == 0), stop=False)
                    nc.tensor.matmul(
                        psv[:], ones_r[0:1, 0:128], bv_f[0:1, :],
                        start=False, stop=True)
                    nc.scalar.copy(
                        V[:, st, :, 0:HD],
                        psv[:].rearrange("p (h e) -> p h e", h=HEADS))

            # ---------- attention: one head pair, both q halves ----------
            def attention(hp):
                heads = (2 * hp, 2 * hp + 1)
                for qh in range(2):
                    attT = {}
                    pso = {}
                    for i, h in enumerate(heads):
                        attT[h] = att_pool.tile(
                            [128, 8, 512], bf16, name=f"attT{h}_{qh}",
                            tag=f"attT{i}")
                        pso[h] = ps_mix.tile([HD + 1, 512], f32,
                                             name=f"pso{h}_{qh}", tag="mix")
                    # pipelined: scores group -> exp -> attV chunk, per ktp
                    for ktp in range(4):
                        pss = {}
                        for i, h in enumerate(heads):
                            pss[h] = ps_s.tile([128, 2, 512], f32,
                                               name=f"pss{h}_{qh}_{ktp}",
                                               tag="pss")
                        for kt2 in range(2):
                            kt = ktp * 2 + kt2
                            for h in heads:
                                po = (h % 2) * HD
                                nc.tensor.matmul(
                                    pss[h][:, kt2, :],
                                    KT[po:po + HD, hp, kt * 128:(kt + 1) * 128],
                                    QT[po:po + HD, hp, qh * 512:(qh + 1) * 512],
                                    start=True, stop=True)
                        for h in heads:
                            nc.scalar.activation(
                                attT[h][:, ktp * 2:ktp * 2 + 2, :],
                                pss[h][:], Exp, scale=0.125)
                        for kc in (ktp * 2, ktp * 2 + 1):
                            for h in heads:
                                nc.tensor.matmul(
                                    pso[h][:],
                                    V[:, kc, h, :],
                                    attT[h][:, kc, :],
                                    start=(kc == 0), stop=(kc == 7))
                    for h in heads:
                        oT = ot_pool.tile([HD + 1, 512], f32, tag="oT")
                        nc.vector.tensor_copy(oT[:], pso[h][:])
                        pbt = ps_mix.tile([128, 4, HD + 1], f32, tag="mix",
                                         name=f"pbt{h}_{qh}")
                        for qs in range(4):
                            nc.tensor.transpose(
                                pbt[:, qs, :],
                                oT[:, qs * 128:(qs + 1) * 128],
                                ident_f32[0:HD + 1, 0:HD + 1])
                        rec = ot_pool.tile([128, 4], f32, tag="rec")
                        nc.vector.reciprocal(rec[:], pbt[:, :, HD])
                        for qs in range(4):
                            qt = qh * 4 + qs
                            nc.vector.tensor_scalar_mul(
                                o_stage[:, qt, h * HD:(h + 1) * HD],
                                pbt[:, qs, 0:HD],
                                rec[:, qs:qs + 1])

            # ---------- emission: V, K, then Q with attention filling ----
            w_v, _, xT_v = load_and_transpose("v")
            proj_v(w_v, xT_v)

            w_k, b_k, xT_k = load_and_transpose("k")
            for dt in range(4):
                proj_qk(KT, w_k, b_k, xT_k, dt)

            w_q, b_q, xT_q = load_and_transpose("q")
            proj_qk(QT, w_q, b_q, xT_q, 0)
            proj_qk(QT, w_q, b_q, xT_q, 1)
            attention(0)
            proj_qk(QT, w_q, b_q, xT_q, 2)
            attention(1)
            proj_qk(QT, w_q, b_q, xT_q, 3)
            attention(2)
            attention(3)

            # ---------------- Phase C: output ----------------
            nc.sync.dma_start(
                out_d[:].rearrange("(t p) d -> p t d", p=128), o_stage[:])

    nc.compile()
    return nc


_NC = None


def _get_nc():
    global _NC
    if _NC is None:
        _NC = build_nc()
    return _NC


def _make_in_maps(inputs):
    in_maps = []
    for b in range(B):
        m = {
            "q_in": np.ascontiguousarray(inputs["q_in"][b].reshape(S, C)),
            "k_in": np.ascontiguousarray(inputs["k_in"][b].reshape(S, C)),
            "v_in": np.ascontiguousarray(inputs["v_in"][b].reshape(S, C)),
            "Wq": np.asarray(inputs["Wq"]), "bq": np.asarray(inputs["bq"]),
            "Wk": np.asarray(inputs["Wk"]), "bk": np.asarray(inputs["bk"]),
            "Wv": np.asarray(inputs["Wv"]), "bv": np.asarray(inputs["bv"]),
        }
        in_maps.append(m)
    return in_maps


def kernel(**inputs):
    nc = _get_nc()
    res = run_bass_kernel_spmd(nc, _make_in_maps(inputs), list(range(N_CORES)))
    out = np.stack([res.results[i]["out"] for i in range(B)])
    return out.reshape(B, HS, WS, D).astype(np.float32)


if __name__ == "__main__":
    rng = np.random.default_rng(0)
    ins = {
        "q_in": rng.standard_normal((B, HS, WS, C), dtype=np.float32),
        "k_in": rng.standard_normal((B, HS, WS, C), dtype=np.float32),
        "v_in": rng.standard_normal((B, HS, WS, C), dtype=np.float32),
        "Wq": (rng.standard_normal((C, D)) / np.sqrt(C)).astype(np.float32),
        "Wk": (rng.standard_normal((C, D)) / np.sqrt(C)).astype(np.float32),
        "Wv": (rng.standard_normal((C, D)) / np.sqrt(C)).astype(np.float32),
        "bq": np.zeros(D, np.float32),
        "bk": np.zeros(D, np.float32),
        "bv": np.zeros(D, np.float32),
    }
    out = kernel(**ins)
    print("out shape:", out.shape, "finite:", np.isfinite(out).all())
